# revision 2
# baseline (speedup 1.0000x reference)
"""Trainium2 Bass kernel for CombinedBandPassFilterSequential.

Zero-phase (filtfilt-style) two-pass FIR filter bank: 10 phase bands
(K=769) + 10 amplitude bands (K=129) over a single (1,1,2097152) fp32
signal; output is the 20 band signals concatenated on the last axis.

Strategy
--------
Time-sharded SPMD over 8 NeuronCores: each core processes a contiguous
T/8 slice of the signal for ALL 20 bands (perfect load balance; the
band axis is only 20 wide and pha/amp bands have 6x different cost).

Per core, each 1-D correlation is cast as a sequence of 128x128 @ 128xN
tensor-engine matmuls using banded-Toeplitz weight chunks:

  out[128*i + r] = sum_q  W_q[:, r] . x_cols[:, i + q - Q0]

where x_cols[p, m] = x[128*m + p] is the signal in "transposed" column
layout (prepared on host) and W_q[p, r] = h[128*(q-Q0) + p - r + c].
The contraction (partition) dim is the tap offset; the moving dim N
packs 412-512 consecutive output blocks, so the PE runs dense
[128,128]x[128,512] matmuls at full rate in float32r.

Pass 1 computes y1 = corr(x, h) with a halo; pass 2 computes
corr(y1, flip(h)) == flip(corr(flip(y1), h)) — exactly the reference's
zero-phase scheme, including the 'SAME' zero-pad crop of y1 at the
global sequence edges (enforced via per-core 0/1 masks on the y1 halo
columns, so one SPMD program serves all cores).
"""
import numpy as np

import concourse.bass as bass
import concourse.tile as tile
from concourse import bacc, mybir
from concourse import bass_utils

# ---- problem geometry (hardcoded per contest rules) ----
T = 2097152
NCORES = 8
L = T // NCORES          # 262144 samples per core
LC = L // 128            # 2048 output columns per core
XH = 8                   # x halo columns each side
XC = LC + 2 * XH         # 2064 x columns
YH = 4                   # y1 halo columns each side
YC = LC + 2 * YH         # 2056 y1 columns
NB = 10                  # bands per filter group
KP, QP, Q0P = 769, 7, 3  # pha: taps, Toeplitz chunks, chunk offset
KA, QA, Q0A = 129, 3, 1  # amp
P1N = 412                # pass-1 matmul moving width (ceil(2056/5))
P2N = 512                # pass-2 matmul moving width

F32 = mybir.dt.float32
F32R = mybir.dt.float32r


def _toeplitz_chunks(h, Q0, NQ):
    """W[q][p, r] = h[128*(q - Q0) + p - r + c], zero outside [0, K)."""
    K = len(h)
    c = (K - 1) // 2
    W = np.zeros((NQ, 128, 128), np.float32)
    p = np.arange(128)[:, None]
    r = np.arange(128)[None, :]
    for q in range(NQ):
        k = 128 * (q - Q0) + p - r + c
        valid = (k >= 0) & (k < K)
        W[q][valid] = h[np.clip(k, 0, K - 1)][valid]
    return W


def _emit_pass1(nc, pools, wt, b, nq, q0, y1):
    """pass 1: y1_ext cols [0, YC); x col m = j + q + (XH - YH - q0)."""
    psum_pool, _ = pools
    xt = wt["x"]
    w1 = wt["w1"]
    for g in range(5):
        j0 = g * P1N
        n = min(P1N, YC - j0)
        ps = psum_pool.tile([128, P2N], F32, tag="ps")
        for q in range(nq):
            m0 = j0 + q + XH - YH - q0
            nc.tensor.matmul(
                ps[:, :n],
                w1[:, (b * nq + q) * 128:(b * nq + q + 1) * 128],
                xt[:, m0:m0 + n],
                start=(q == 0), stop=(q == nq - 1),
            )
        nc.vector.tensor_copy(y1[:, j0:j0 + n], ps[:, :n])
    # 'SAME' crop of y1 outside the global [0, T) range (masks are 0/1
    # per core; inner cores get all-ones)
    nc.vector.tensor_mul(y1[:, :YH], y1[:, :YH], wt["mask_l"][:])
    nc.vector.tensor_mul(y1[:, YC - YH:], y1[:, YC - YH:], wt["mask_r"][:])


def _emit_pass2(nc, pools, wt, b, nq, q0, y1, out_ap, ob):
    """pass 2: out col i reads y1 cols j = i + q - q0 + YH."""
    psum_pool, stage_pool = pools
    w2 = wt["w2"]
    for g in range(LC // P2N):
        i0 = g * P2N
        ps = psum_pool.tile([128, P2N], F32, tag="ps")
        for q in range(nq):
            j0 = i0 + q - q0 + YH
            nc.tensor.matmul(
                ps[:],
                w2[:, (b * nq + q) * 128:(b * nq + q + 1) * 128],
                y1[:, j0:j0 + P2N],
                start=(q == 0), stop=(q == nq - 1),
            )
        st = stage_pool.tile([128, P2N], F32, tag="st")
        nc.vector.tensor_copy(st[:], ps[:])
        nc.sync.dma_start(out_ap[ob, :, i0:i0 + P2N], st[:])


def _build_program():
    nc = bacc.Bacc("TRN2", target_bir_lowering=False, debug=False,
                   enable_asserts=True, num_devices=NCORES)

    x_ap = nc.dram_tensor("xT", [128, XC], F32R, kind="ExternalInput").ap()
    wp1_ap = nc.dram_tensor("wp1", [128, NB * QP * 128], F32R,
                            kind="ExternalInput").ap()
    wp2_ap = nc.dram_tensor("wp2", [128, NB * QP * 128], F32R,
                            kind="ExternalInput").ap()
    wa1_ap = nc.dram_tensor("wa1", [128, NB * QA * 128], F32R,
                            kind="ExternalInput").ap()
    wa2_ap = nc.dram_tensor("wa2", [128, NB * QA * 128], F32R,
                            kind="ExternalInput").ap()
    ml_ap = nc.dram_tensor("mask_l", [128, YH], F32R, kind="ExternalInput").ap()
    mr_ap = nc.dram_tensor("mask_r", [128, YH], F32R, kind="ExternalInput").ap()
    out_ap = nc.dram_tensor("out", [2 * NB, 128, LC], F32,
                            kind="ExternalOutput").ap()

    with tile.TileContext(nc) as tc:
        with tc.tile_pool(name="const", bufs=1) as cpool, \
             tc.tile_pool(name="y1", bufs=2) as y1_pool, \
             tc.tile_pool(name="psum", bufs=8, space="PSUM") as psum_pool, \
             tc.tile_pool(name="stage", bufs=4) as stage_pool:

            wt = {
                "x": cpool.tile([128, XC], F32R, name="xt", tag="x"),
                "wp1": cpool.tile([128, NB * QP * 128], F32R, name="wp1t",
                                  tag="wp1"),
                "wp2": cpool.tile([128, NB * QP * 128], F32R, name="wp2t",
                                  tag="wp2"),
                "wa1": cpool.tile([128, NB * QA * 128], F32R, name="wa1t",
                                  tag="wa1"),
                "wa2": cpool.tile([128, NB * QA * 128], F32R, name="wa2t",
                                  tag="wa2"),
                "mask_l": cpool.tile([128, YH], F32R, name="mlt",
                                     tag="mask_l"),
                "mask_r": cpool.tile([128, YH], F32R, name="mrt",
                                     tag="mask_r"),
            }
            nc.sync.dma_start(wt["x"][:], x_ap[:])
            nc.sync.dma_start(wt["wp1"][:], wp1_ap[:])
            nc.sync.dma_start(wt["wp2"][:], wp2_ap[:])
            nc.sync.dma_start(wt["wa1"][:], wa1_ap[:])
            nc.sync.dma_start(wt["wa2"][:], wa2_ap[:])
            nc.sync.dma_start(wt["mask_l"][:], ml_ap[:])
            nc.sync.dma_start(wt["mask_r"][:], mr_ap[:])

            pools = (psum_pool, stage_pool)
            # band schedule: 0-9 pha, 10-19 amp; emit pass1(b+1) before
            # pass2(b) so the PE never waits on the pass1->pass2 handoff
            bands = []
            for b in range(NB):
                bands.append((b, QP, Q0P, "wp1", "wp2", b))
            for b in range(NB):
                bands.append((b, QA, Q0A, "wa1", "wa2", NB + b))

            y1_tiles = [None] * len(bands)

            def pass1(idx):
                b, nq, q0, w1k, w2k, ob = bands[idx]
                y1 = y1_pool.tile([128, YC], F32R, tag="y1")
                y1_tiles[idx] = y1
                wtb = dict(wt)
                wtb["w1"] = wt[w1k]
                _emit_pass1(nc, pools, wtb, b, nq, q0, y1)

            def pass2(idx):
                b, nq, q0, w1k, w2k, ob = bands[idx]
                wtb = dict(wt)
                wtb["w2"] = wt[w2k]
                _emit_pass2(nc, pools, wtb, b, nq, q0, y1_tiles[idx],
                            out_ap, ob)

            pass1(0)
            for i in range(len(bands)):
                if i + 1 < len(bands):
                    pass1(i + 1)
                pass2(i)

    nc.compile()
    return nc


_CACHE = {}


def _get_program():
    if "nc" not in _CACHE:
        _CACHE["nc"] = _build_program()
    return _CACHE["nc"]


def _host_inputs(x, pha_filters, amp_filters):
    x = np.ascontiguousarray(np.asarray(x, np.float32).reshape(T))
    pha = np.asarray(pha_filters, np.float32)
    amp = np.asarray(amp_filters, np.float32)

    wp1 = np.stack([_toeplitz_chunks(h, Q0P, QP) for h in pha])
    wp2 = np.stack([_toeplitz_chunks(h[::-1], Q0P, QP) for h in pha])
    wa1 = np.stack([_toeplitz_chunks(h, Q0A, QA) for h in amp])
    wa2 = np.stack([_toeplitz_chunks(h[::-1], Q0A, QA) for h in amp])

    def wlay(W):  # (NB, NQ, 128p, 128r) -> (128p, NB*NQ*128r)
        return np.ascontiguousarray(
            W.transpose(2, 0, 1, 3).reshape(128, -1))

    wp1, wp2, wa1, wa2 = wlay(wp1), wlay(wp2), wlay(wa1), wlay(wa2)

    xp = np.zeros(T + 2 * XH * 128, np.float32)
    xp[XH * 128: XH * 128 + T] = x

    ones = np.ones((128, YH), np.float32)
    zeros = np.zeros((128, YH), np.float32)

    in_maps = []
    for c in range(NCORES):
        n0 = c * L
        xT = np.ascontiguousarray(
            xp[n0:n0 + L + 2 * XH * 128].reshape(XC, 128).T)
        in_maps.append({
            "xT": xT,
            "wp1": wp1, "wp2": wp2, "wa1": wa1, "wa2": wa2,
            "mask_l": zeros if c == 0 else ones,
            "mask_r": zeros if c == NCORES - 1 else ones,
        })
    return in_maps


def _gather(results):
    out = np.empty((2 * NB, T), np.float32)
    for c in range(NCORES):
        oc = results[c]["out"]  # (20, 128, LC): [band, r, i] = y[128*i + r]
        out[:, c * L:(c + 1) * L] = oc.transpose(0, 2, 1).reshape(2 * NB, L)
    return out.reshape(1, 1, 2 * NB * T)


def run(x, pha_filters, amp_filters, trace=False):
    nc = _get_program()
    in_maps = _host_inputs(x, pha_filters, amp_filters)
    res = bass_utils.run_bass_kernel_spmd(
        nc, in_maps, core_ids=list(range(NCORES)), trace=trace)
    return _gather(res.results), res


def kernel(x, pha_filters, amp_filters):
    out, _ = run(x, pha_filters, amp_filters)
    return out


# revision 4
# speedup vs baseline: 1.1777x; 1.1777x over previous
"""Trainium2 Bass kernel for CombinedBandPassFilterSequential.

Zero-phase (filtfilt-style) FIR filter bank: 10 phase bands (K=769) +
10 amplitude bands (K=129) over a single (1,1,2097152) fp32 signal;
output is the 20 band signals concatenated on the last axis.

Strategy
--------
Time-sharded SPMD over 8 NeuronCores: each core processes a contiguous
T/8 slice of the signal for ALL 20 bands (perfect load balance).

Each 1-D correlation is cast as a sequence of 128x128 @ 128xN
tensor-engine matmuls using banded-Toeplitz weight chunks:

  out[128*i + r] = sum_q  W_q[:, r] . x_cols[:, i + q - Q0]

where x_cols[p, m] = x[128*m + p] is the signal in "transposed" column
layout (prepared on host) and W_q[p, r] = h[128*(q-Q0) + p - r + c].
The contraction (partition) dim is the tap offset; the moving dim packs
412-512 consecutive output blocks, so the PE runs dense matmuls at full
rate in float32r.

pha bands: two passes (corr with h, then with flip(h)) — exactly the
reference's zero-phase scheme; the 'SAME' zero-pad crop of the
intermediate at the global sequence edges is enforced via per-core 0/1
masks on its halo columns (one SPMD program serves all cores).

amp bands: single fused pass with g = autocorr(h) (257 taps), which
equals the two-pass result everywhere except the first/last 64 samples
of the GLOBAL sequence; those get an exact rank-64 correction
(precomputed 64x64 matrices applied to x's global head/tail, fed as a
per-core input that is zero except on cores 0/7 — again SPMD-uniform).
"""
import numpy as np

import concourse.bass as bass
import concourse.tile as tile
from concourse import bacc, mybir
from concourse import bass_utils

# ---- problem geometry (hardcoded per contest rules) ----
T = 2097152
NCORES = 8
L = T // NCORES          # 262144 samples per core
LC = L // 128            # 2048 output columns per core
XH = 8                   # x halo columns each side
XC = LC + 2 * XH         # 2064 x columns
YH = 4                   # y1 halo columns each side (pha)
YC = LC + 2 * YH         # 2056 y1 columns
NB = 10                  # bands per filter group
KP, QP, Q0P = 769, 7, 3  # pha: taps, Toeplitz chunks, chunk offset
KA = 129                 # amp taps
QG, Q0G = 3, 1           # fused amp autocorr (257 taps): chunks, offset
CA = (KA - 1) // 2       # 64: amp edge-correction width
P1N = 412                # pass-1 matmul moving width (ceil(2056/5))
P2N = 512                # pass-2 / fused matmul moving width

F32 = mybir.dt.float32
F32R = mybir.dt.float32r


def _toeplitz_chunks(h, Q0, NQ):
    """W[q][p, r] = h[128*(q - Q0) + p - r + c], zero outside [0, K)."""
    h = np.asarray(h, np.float64)
    K = len(h)
    c = (K - 1) // 2
    W = np.zeros((NQ, 128, 128), np.float64)
    p = np.arange(128)[:, None]
    r = np.arange(128)[None, :]
    for q in range(NQ):
        k = 128 * (q - Q0) + p - r + c
        valid = (k >= 0) & (k < K)
        W[q][valid] = h[np.clip(k, 0, K - 1)][valid]
    return W.astype(np.float32)


def _amp_corr_lhsT(h):
    """Block-diag [MleftT, MrightT] edge-correction matrix for one amp band.

    ref[n] = fused[n] - Mleft[n,:] @ x[:64]        for n in [0, 64)
    ref[n] = fused[n] - Mright[n-T+64,:] @ x[T-64:] for n in [T-64, T)
    """
    h = np.asarray(h, np.float64)
    K = len(h)
    c = (K - 1) // 2

    def hpad(idx):
        v = np.zeros(idx.shape)
        ok = (idx >= 0) & (idx < K)
        v[ok] = h[np.clip(idx, 0, K - 1)][ok]
        return v

    n = np.arange(c)[:, None, None]
    i = np.arange(c)[None, :, None]
    m = np.arange(-c, 0)[None, None, :]
    Mleft = (hpad(n + c - m) * hpad(i + c - m)).sum(-1)
    mm = np.arange(1, c + 1)[None, None, :]
    Mright = (hpad(n - mm + 1) * hpad(i - mm + 1)).sum(-1)

    lhsT = np.zeros((128, 128), np.float64)
    lhsT[:c, :c] = Mleft.T          # lhsT[p=i, r=n]
    lhsT[c:2 * c, c:2 * c] = Mright.T
    return lhsT.astype(np.float32)


def _build_program():
    nc = bacc.Bacc("TRN2", target_bir_lowering=False, debug=False,
                   enable_asserts=True, num_devices=NCORES)

    x_ap = nc.dram_tensor("xT", [128, XC], F32R, kind="ExternalInput").ap()
    wp1_ap = nc.dram_tensor("wp1", [128, NB * QP * 128], F32R,
                            kind="ExternalInput").ap()
    wp2_ap = nc.dram_tensor("wp2", [128, NB * QP * 128], F32R,
                            kind="ExternalInput").ap()
    wg_ap = nc.dram_tensor("wg", [128, NB * QG * 128], F32R,
                           kind="ExternalInput").ap()
    wc_ap = nc.dram_tensor("wc", [128, NB * 128], F32,
                           kind="ExternalInput").ap()
    xe_ap = nc.dram_tensor("xe", [128, 1], F32, kind="ExternalInput").ap()
    ml_ap = nc.dram_tensor("mask_l", [128, YH], F32R, kind="ExternalInput").ap()
    mr_ap = nc.dram_tensor("mask_r", [128, YH], F32R, kind="ExternalInput").ap()
    out_ap = nc.dram_tensor("out", [2 * NB, 128, LC], F32,
                            kind="ExternalOutput").ap()

    with tile.TileContext(nc) as tc:
        with tc.tile_pool(name="const", bufs=1) as cpool, \
             tc.tile_pool(name="y1", bufs=2) as y1_pool, \
             tc.tile_pool(name="psum", bufs=6, space="PSUM") as psum_pool, \
             tc.tile_pool(name="psumc", bufs=2, space="PSUM") as psumc_pool, \
             tc.tile_pool(name="stage", bufs=4) as stage_pool:

            xt = cpool.tile([128, XC], F32R, name="xt", tag="x")
            wp1 = cpool.tile([128, NB * QP * 128], F32R, name="wp1t", tag="wp1")
            wp2 = cpool.tile([128, NB * QP * 128], F32R, name="wp2t", tag="wp2")
            wg = cpool.tile([128, NB * QG * 128], F32R, name="wgt", tag="wg")
            wc = cpool.tile([128, NB * 128], F32, name="wct", tag="wc")
            xe = cpool.tile([128, 1], F32, name="xet", tag="xe")
            ml = cpool.tile([128, YH], F32R, name="mlt", tag="mask_l")
            mr = cpool.tile([128, YH], F32R, name="mrt", tag="mask_r")

            # DMAs ordered by first use: x + band-0 weights + masks first,
            # so the PE never waits on the bulk of the weight traffic.
            nc.sync.dma_start(xt[:], x_ap[:])

            def wslice(tile_, ap, b, nq):
                s = b * nq * 128
                e = (b + 1) * nq * 128
                nc.sync.dma_start(tile_[:, s:e], ap[:, s:e])

            wslice(wp1, wp1_ap, 0, QP)
            nc.sync.dma_start(ml[:], ml_ap[:])
            nc.sync.dma_start(mr[:], mr_ap[:])
            wslice(wp2, wp2_ap, 0, QP)
            for b in range(1, NB):
                wslice(wp1, wp1_ap, b, QP)
                wslice(wp2, wp2_ap, b, QP)
            nc.sync.dma_start(xe[:], xe_ap[:])
            for b in range(NB):
                wslice(wg, wg_ap, b, QG)
                wslice(wc, wc_ap, b, 1)

            y1_tiles = [None] * NB

            def pha_pass1(b):
                y1 = y1_pool.tile([128, YC], F32R, tag="y1")
                y1_tiles[b] = y1
                for g in range(5):
                    j0 = g * P1N
                    n = min(P1N, YC - j0)
                    ps = psum_pool.tile([128, P2N], F32, tag="ps")
                    for q in range(QP):
                        m0 = j0 + q + XH - YH - Q0P
                        nc.tensor.matmul(
                            ps[:, :n],
                            wp1[:, (b * QP + q) * 128:(b * QP + q + 1) * 128],
                            xt[:, m0:m0 + n],
                            start=(q == 0), stop=(q == QP - 1),
                        )
                    nc.vector.tensor_copy(y1[:, j0:j0 + n], ps[:, :n])
                # 'SAME' crop of y1 outside the global [0, T) range
                nc.vector.tensor_mul(y1[:, :YH], y1[:, :YH], ml[:])
                nc.vector.tensor_mul(y1[:, YC - YH:], y1[:, YC - YH:], mr[:])

            def pha_pass2(b):
                y1 = y1_tiles[b]
                for g in range(LC // P2N):
                    i0 = g * P2N
                    ps = psum_pool.tile([128, P2N], F32, tag="ps")
                    for q in range(QP):
                        j0 = i0 + q - Q0P + YH
                        nc.tensor.matmul(
                            ps[:],
                            wp2[:, (b * QP + q) * 128:(b * QP + q + 1) * 128],
                            y1[:, j0:j0 + P2N],
                            start=(q == 0), stop=(q == QP - 1),
                        )
                    st = stage_pool.tile([128, P2N], F32, tag="st")
                    nc.vector.tensor_copy(st[:], ps[:])
                    nc.sync.dma_start(out_ap[b, :, i0:i0 + P2N], st[:])

            def amp_band(b):
                # rank-64 global-edge correction (zero on inner cores)
                pc = psumc_pool.tile([128, 1], F32, tag="pc")
                nc.tensor.matmul(pc[:], wc[:, b * 128:(b + 1) * 128], xe[:],
                                 start=True, stop=True)
                for g in range(LC // P2N):
                    i0 = g * P2N
                    ps = psum_pool.tile([128, P2N], F32, tag="ps")
                    for q in range(QG):
                        m0 = i0 + q - Q0G + XH
                        nc.tensor.matmul(
                            ps[:],
                            wg[:, (b * QG + q) * 128:(b * QG + q + 1) * 128],
                            xt[:, m0:m0 + P2N],
                            start=(q == 0), stop=(q == QG - 1),
                        )
                    st = stage_pool.tile([128, P2N], F32, tag="st")
                    nc.vector.tensor_copy(st[:], ps[:])
                    if g == 0:
                        nc.vector.tensor_sub(st[:CA, :1], st[:CA, :1],
                                             pc[:CA, :])
                    if g == LC // P2N - 1:
                        nc.vector.tensor_sub(st[CA:2 * CA, P2N - 1:],
                                             st[CA:2 * CA, P2N - 1:],
                                             pc[CA:2 * CA, :])
                    nc.sync.dma_start(out_ap[NB + b, :, i0:i0 + P2N], st[:])

            # pha bands software-pipelined so the PE never waits on the
            # pass1 -> pass2 handoff; amp bands have no handoff.
            pha_pass1(0)
            for b in range(NB):
                if b + 1 < NB:
                    pha_pass1(b + 1)
                pha_pass2(b)
            for b in range(NB):
                amp_band(b)

    nc.compile()
    return nc


_CACHE = {}


def _get_program():
    if "nc" not in _CACHE:
        _CACHE["nc"] = _build_program()
    return _CACHE["nc"]


def _host_inputs(x, pha_filters, amp_filters):
    x = np.ascontiguousarray(np.asarray(x, np.float32).reshape(T))
    pha = np.asarray(pha_filters, np.float32)
    amp = np.asarray(amp_filters, np.float32)

    wp1 = np.stack([_toeplitz_chunks(h, Q0P, QP) for h in pha])
    wp2 = np.stack([_toeplitz_chunks(h[::-1], Q0P, QP) for h in pha])
    gs = [np.correlate(np.asarray(h, np.float64),
                       np.asarray(h, np.float64), "full") for h in amp]
    wg = np.stack([_toeplitz_chunks(g, Q0G, QG) for g in gs])
    wc = np.stack([_amp_corr_lhsT(h) for h in amp])  # (NB, 128, 128)

    def wlay(W):  # (NB, NQ, 128p, 128r) -> (128p, NB*NQ*128r)
        return np.ascontiguousarray(W.transpose(2, 0, 1, 3).reshape(128, -1))

    wp1, wp2, wg = wlay(wp1), wlay(wp2), wlay(wg)
    wc = np.ascontiguousarray(wc.transpose(1, 0, 2).reshape(128, -1))

    xp = np.zeros(T + 2 * XH * 128, np.float32)
    xp[XH * 128: XH * 128 + T] = x

    ones = np.ones((128, YH), np.float32)
    zeros = np.zeros((128, YH), np.float32)
    xe0 = np.zeros((128, 1), np.float32)
    xe_head = xe0.copy()
    xe_head[:CA, 0] = x[:CA]
    xe_tail = xe0.copy()
    xe_tail[CA:2 * CA, 0] = x[T - CA:]

    in_maps = []
    for c in range(NCORES):
        n0 = c * L
        xT = np.ascontiguousarray(
            xp[n0:n0 + L + 2 * XH * 128].reshape(XC, 128).T)
        in_maps.append({
            "xT": xT,
            "wp1": wp1, "wp2": wp2, "wg": wg, "wc": wc,
            "xe": xe_head if c == 0 else (xe_tail if c == NCORES - 1 else xe0),
            "mask_l": zeros if c == 0 else ones,
            "mask_r": zeros if c == NCORES - 1 else ones,
        })
    return in_maps


def _gather(results):
    out = np.empty((2 * NB, T), np.float32)
    for c in range(NCORES):
        oc = results[c]["out"]  # (20, 128, LC): [band, r, i] = y[128*i + r]
        out[:, c * L:(c + 1) * L] = oc.transpose(0, 2, 1).reshape(2 * NB, L)
    return out.reshape(1, 1, 2 * NB * T)


def run(x, pha_filters, amp_filters, trace=False):
    nc = _get_program()
    in_maps = _host_inputs(x, pha_filters, amp_filters)
    res = bass_utils.run_bass_kernel_spmd(
        nc, in_maps, core_ids=list(range(NCORES)), trace=trace)
    return _gather(res.results), res


def kernel(x, pha_filters, amp_filters):
    out, _ = run(x, pha_filters, amp_filters)
    return out


# revision 7
# speedup vs baseline: 1.2120x; 1.0292x over previous
"""Trainium2 Bass kernel for CombinedBandPassFilterSequential.

Zero-phase (filtfilt-style) FIR filter bank: 10 phase bands (K=769) +
10 amplitude bands (K=129) over a single (1,1,2097152) fp32 signal;
output is the 20 band signals concatenated on the last axis.

Strategy
--------
Time-sharded SPMD over 8 NeuronCores: each core processes a contiguous
T/8 slice of the signal for ALL 20 bands (perfect load balance).

Each 1-D correlation is cast as a sequence of 128x128 @ 128xN
tensor-engine matmuls using banded-Toeplitz weight chunks:

  out[128*i + r] = sum_q  W_q[:, r] . x_cols[:, i + q - Q0]

where x_cols[p, m] = x[128*m + p] is the signal in "transposed" column
layout (prepared on host) and W_q[p, r] = h[128*(q-Q0) + p - r + c].
The contraction (partition) dim is the tap offset; the moving dim packs
412-512 consecutive output blocks, so the PE runs dense matmuls at full
rate in float32r.

pha bands: two passes (corr with h, then with flip(h)) — exactly the
reference's zero-phase scheme; the 'SAME' zero-pad crop of the
intermediate at the global sequence edges is enforced via per-core 0/1
masks on its halo columns (one SPMD program serves all cores).

amp bands: single fused pass with g = autocorr(h) (257 taps), which
equals the two-pass result everywhere except the first/last 64 samples
of the GLOBAL sequence; those get an exact rank-64 correction
(precomputed 64x64 matrices applied to x's global head/tail, fed as a
per-core input that is zero except on cores 0/7 — again SPMD-uniform).
"""
import numpy as np

import concourse.bass as bass
import concourse.tile as tile
from concourse import bacc, mybir
from concourse import bass_utils

# ---- problem geometry (hardcoded per contest rules) ----
T = 2097152
NCORES = 8
L = T // NCORES          # 262144 samples per core
LC = L // 128            # 2048 output columns per core
XH = 8                   # x halo columns each side
XC = LC + 2 * XH         # 2064 x columns
YH = 4                   # y1 halo columns each side (pha)
YC = LC + 2 * YH         # 2056 y1 columns
NB = 10                  # bands per filter group
KP, QP, Q0P = 769, 7, 3  # pha: taps, Toeplitz chunks, chunk offset
KA = 129                 # amp taps
QG, Q0G = 3, 1           # fused amp autocorr (257 taps): chunks, offset
CA = (KA - 1) // 2       # 64: amp edge-correction width
P1N = 412                # pass-1 matmul moving width (ceil(2056/5))
P2N = 512                # pass-2 / fused matmul moving width

F32 = mybir.dt.float32
F32R = mybir.dt.float32r


def _toeplitz_chunks(h, Q0, NQ):
    """W[q][p, r] = h[128*(q - Q0) + p - r + c], zero outside [0, K)."""
    h = np.asarray(h, np.float64)
    K = len(h)
    c = (K - 1) // 2
    W = np.zeros((NQ, 128, 128), np.float64)
    p = np.arange(128)[:, None]
    r = np.arange(128)[None, :]
    for q in range(NQ):
        k = 128 * (q - Q0) + p - r + c
        valid = (k >= 0) & (k < K)
        W[q][valid] = h[np.clip(k, 0, K - 1)][valid]
    return W.astype(np.float32)


def _amp_corr_lhsT(h):
    """Block-diag [MleftT, MrightT] edge-correction matrix for one amp band.

    ref[n] = fused[n] - Mleft[n,:] @ x[:64]        for n in [0, 64)
    ref[n] = fused[n] - Mright[n-T+64,:] @ x[T-64:] for n in [T-64, T)
    """
    h = np.asarray(h, np.float64)
    K = len(h)
    c = (K - 1) // 2

    def hpad(idx):
        v = np.zeros(idx.shape)
        ok = (idx >= 0) & (idx < K)
        v[ok] = h[np.clip(idx, 0, K - 1)][ok]
        return v

    n = np.arange(c)[:, None, None]
    i = np.arange(c)[None, :, None]
    m = np.arange(-c, 0)[None, None, :]
    Mleft = (hpad(n + c - m) * hpad(i + c - m)).sum(-1)
    mm = np.arange(1, c + 1)[None, None, :]
    Mright = (hpad(n - mm + 1) * hpad(i - mm + 1)).sum(-1)

    lhsT = np.zeros((128, 128), np.float64)
    lhsT[:c, :c] = Mleft.T          # lhsT[p=i, r=n]
    lhsT[c:2 * c, c:2 * c] = Mright.T
    return lhsT.astype(np.float32)


def _build_program():
    nc = bacc.Bacc("TRN2", target_bir_lowering=False, debug=False,
                   enable_asserts=True, num_devices=NCORES)

    x_ap = nc.dram_tensor("xT", [128, XC], F32R, kind="ExternalInput").ap()
    wp1_ap = nc.dram_tensor("wp1", [128, NB * QP * 128], F32R,
                            kind="ExternalInput").ap()
    wp2_ap = nc.dram_tensor("wp2", [128, NB * QP * 128], F32R,
                            kind="ExternalInput").ap()
    wg_ap = nc.dram_tensor("wg", [128, NB * QG * 128], F32R,
                           kind="ExternalInput").ap()
    wc_ap = nc.dram_tensor("wc", [128, NB * 128], F32,
                           kind="ExternalInput").ap()
    xe_ap = nc.dram_tensor("xe", [128, 1], F32, kind="ExternalInput").ap()
    ml_ap = nc.dram_tensor("mask_l", [128, YH], F32R, kind="ExternalInput").ap()
    mr_ap = nc.dram_tensor("mask_r", [128, YH], F32R, kind="ExternalInput").ap()
    out_ap = nc.dram_tensor("out", [2 * NB, 128, LC], F32,
                            kind="ExternalOutput").ap()

    with tile.TileContext(nc) as tc:
        with tc.tile_pool(name="const", bufs=1) as cpool, \
             tc.tile_pool(name="y1", bufs=2) as y1_pool, \
             tc.tile_pool(name="psum", bufs=6, space="PSUM") as psum_pool, \
             tc.tile_pool(name="psumc", bufs=2, space="PSUM") as psumc_pool, \
             tc.tile_pool(name="stage", bufs=4) as stage_pool:

            xt = cpool.tile([128, XC], F32R, name="xt", tag="x")
            wp1 = cpool.tile([128, NB * QP * 128], F32R, name="wp1t", tag="wp1")
            wp2 = cpool.tile([128, NB * QP * 128], F32R, name="wp2t", tag="wp2")
            wg = cpool.tile([128, NB * QG * 128], F32R, name="wgt", tag="wg")
            wc = cpool.tile([128, NB * 128], F32, name="wct", tag="wc")
            xe = cpool.tile([128, 1], F32, name="xet", tag="xe")
            ml = cpool.tile([128, YH], F32R, name="mlt", tag="mask_l")
            mr = cpool.tile([128, YH], F32R, name="mrt", tag="mask_r")

            # DMAs ordered by first use: x + band-0 weights + masks first,
            # so the PE never waits on the bulk of the weight traffic.
            # xT lands in 4 chunks so pass1(0) group 0 only waits for the
            # columns it reads.
            # xT on the ACT HWDGE ring, weights on the SP ring — the two
            # rings drain in parallel, halving time-to-first-matmul
            for s in range(0, XC, 516):
                e = min(XC, s + 516)
                nc.scalar.dma_start(xt[:, s:e], x_ap[:, s:e])

            def wslice(tile_, ap, b, nq):
                s = b * nq * 128
                e = (b + 1) * nq * 128
                nc.sync.dma_start(tile_[:, s:e], ap[:, s:e])

            wslice(wp1, wp1_ap, 0, QP)
            nc.sync.dma_start(ml[:], ml_ap[:])
            nc.sync.dma_start(mr[:], mr_ap[:])
            wslice(wp2, wp2_ap, 0, QP)
            for b in range(1, NB):
                wslice(wp1, wp1_ap, b, QP)
                wslice(wp2, wp2_ap, b, QP)
            nc.sync.dma_start(xe[:], xe_ap[:])
            for b in range(NB):
                wslice(wg, wg_ap, b, QG)
                wslice(wc, wc_ap, b, 1)

            y1_tiles = [None] * NB

            def pha_pass1(b):
                y1 = y1_pool.tile([128, YC], F32R, tag="y1")
                y1_tiles[b] = y1
                for g in range(5):
                    j0 = g * P1N
                    n = min(P1N, YC - j0)
                    ps = psum_pool.tile([128, P2N], F32, tag="ps")
                    for q in range(QP):
                        m0 = j0 + q + XH - YH - Q0P
                        nc.tensor.matmul(
                            ps[:, :n],
                            wp1[:, (b * QP + q) * 128:(b * QP + q + 1) * 128],
                            xt[:, m0:m0 + n],
                            start=(q == 0), stop=(q == QP - 1),
                        )
                    nc.vector.tensor_copy(y1[:, j0:j0 + n], ps[:, :n])
                # 'SAME' crop of y1 outside the global [0, T) range
                nc.vector.tensor_mul(y1[:, :YH], y1[:, :YH], ml[:])
                nc.vector.tensor_mul(y1[:, YC - YH:], y1[:, YC - YH:], mr[:])

            def pha_pass2(b):
                y1 = y1_tiles[b]
                for g in range(LC // P2N):
                    i0 = g * P2N
                    ps = psum_pool.tile([128, P2N], F32, tag="ps")
                    for q in range(QP):
                        j0 = i0 + q - Q0P + YH
                        nc.tensor.matmul(
                            ps[:],
                            wp2[:, (b * QP + q) * 128:(b * QP + q + 1) * 128],
                            y1[:, j0:j0 + P2N],
                            start=(q == 0), stop=(q == QP - 1),
                        )
                    st = stage_pool.tile([128, P2N], F32, tag="st")
                    nc.vector.tensor_copy(st[:], ps[:])
                    nc.sync.dma_start(out_ap[b, :, i0:i0 + P2N], st[:])

            def amp_band(b):
                # rank-64 global-edge correction (zero on inner cores)
                pc = psumc_pool.tile([128, 1], F32, tag="pc")
                nc.tensor.matmul(pc[:], wc[:, b * 128:(b + 1) * 128], xe[:],
                                 start=True, stop=True)
                for g in range(LC // P2N):
                    i0 = g * P2N
                    ps = psum_pool.tile([128, P2N], F32, tag="ps")
                    for q in range(QG):
                        m0 = i0 + q - Q0G + XH
                        nc.tensor.matmul(
                            ps[:],
                            wg[:, (b * QG + q) * 128:(b * QG + q + 1) * 128],
                            xt[:, m0:m0 + P2N],
                            start=(q == 0), stop=(q == QG - 1),
                        )
                    st = stage_pool.tile([128, P2N], F32, tag="st")
                    # alternate PSUM->SBUF drains across DVE and ACT so the
                    # copies keep up with the 3-matmul amp groups
                    if g % 2 == 0:
                        nc.vector.tensor_copy(st[:], ps[:])
                    else:
                        nc.scalar.copy(st[:], ps[:])
                    if g == 0:
                        nc.vector.tensor_sub(st[:CA, :1], st[:CA, :1],
                                             pc[:CA, :])
                    if g == LC // P2N - 1:
                        nc.vector.tensor_sub(st[CA:2 * CA, P2N - 1:],
                                             st[CA:2 * CA, P2N - 1:],
                                             pc[CA:2 * CA, :])
                    nc.sync.dma_start(out_ap[NB + b, :, i0:i0 + P2N], st[:])

            # pha bands software-pipelined so the PE never waits on the
            # pass1 -> pass2 handoff; amp bands have no handoff.
            pha_pass1(0)
            for b in range(NB):
                if b + 1 < NB:
                    pha_pass1(b + 1)
                pha_pass2(b)
            for b in range(NB):
                amp_band(b)

    nc.compile()
    return nc


_CACHE = {}


def _get_program():
    if "nc" not in _CACHE:
        _CACHE["nc"] = _build_program()
    return _CACHE["nc"]


def _host_inputs(x, pha_filters, amp_filters):
    x = np.ascontiguousarray(np.asarray(x, np.float32).reshape(T))
    pha = np.asarray(pha_filters, np.float32)
    amp = np.asarray(amp_filters, np.float32)

    wp1 = np.stack([_toeplitz_chunks(h, Q0P, QP) for h in pha])
    wp2 = np.stack([_toeplitz_chunks(h[::-1], Q0P, QP) for h in pha])
    gs = [np.correlate(np.asarray(h, np.float64),
                       np.asarray(h, np.float64), "full") for h in amp]
    wg = np.stack([_toeplitz_chunks(g, Q0G, QG) for g in gs])
    wc = np.stack([_amp_corr_lhsT(h) for h in amp])  # (NB, 128, 128)

    def wlay(W):  # (NB, NQ, 128p, 128r) -> (128p, NB*NQ*128r)
        return np.ascontiguousarray(W.transpose(2, 0, 1, 3).reshape(128, -1))

    wp1, wp2, wg = wlay(wp1), wlay(wp2), wlay(wg)
    wc = np.ascontiguousarray(wc.transpose(1, 0, 2).reshape(128, -1))

    xp = np.zeros(T + 2 * XH * 128, np.float32)
    xp[XH * 128: XH * 128 + T] = x

    ones = np.ones((128, YH), np.float32)
    zeros = np.zeros((128, YH), np.float32)
    xe0 = np.zeros((128, 1), np.float32)
    xe_head = xe0.copy()
    xe_head[:CA, 0] = x[:CA]
    xe_tail = xe0.copy()
    xe_tail[CA:2 * CA, 0] = x[T - CA:]

    in_maps = []
    for c in range(NCORES):
        n0 = c * L
        xT = np.ascontiguousarray(
            xp[n0:n0 + L + 2 * XH * 128].reshape(XC, 128).T)
        in_maps.append({
            "xT": xT,
            "wp1": wp1, "wp2": wp2, "wg": wg, "wc": wc,
            "xe": xe_head if c == 0 else (xe_tail if c == NCORES - 1 else xe0),
            "mask_l": zeros if c == 0 else ones,
            "mask_r": zeros if c == NCORES - 1 else ones,
        })
    return in_maps


def _gather(results):
    out = np.empty((2 * NB, T), np.float32)
    for c in range(NCORES):
        oc = results[c]["out"]  # (20, 128, LC): [band, r, i] = y[128*i + r]
        out[:, c * L:(c + 1) * L] = oc.transpose(0, 2, 1).reshape(2 * NB, L)
    return out.reshape(1, 1, 2 * NB * T)


def run(x, pha_filters, amp_filters, trace=False):
    nc = _get_program()
    in_maps = _host_inputs(x, pha_filters, amp_filters)
    res = bass_utils.run_bass_kernel_spmd(
        nc, in_maps, core_ids=list(range(NCORES)), trace=trace)
    return _gather(res.results), res


def kernel(x, pha_filters, amp_filters):
    out, _ = run(x, pha_filters, amp_filters)
    return out


# revision 10
# speedup vs baseline: 1.2249x; 1.0107x over previous
"""Trainium2 Bass kernel for CombinedBandPassFilterSequential.

Zero-phase (filtfilt-style) FIR filter bank: 10 phase bands (K=769) +
10 amplitude bands (K=129) over a single (1,1,2097152) fp32 signal;
output is the 20 band signals concatenated on the last axis.

Strategy
--------
Time-sharded SPMD over 8 NeuronCores: each core processes a contiguous
T/8 slice of the signal for ALL 20 bands (perfect load balance).

Each 1-D correlation is cast as a sequence of 128x128 @ 128xN
tensor-engine matmuls using banded-Toeplitz weight chunks:

  out[128*i + r] = sum_q  W_q[:, r] . x_cols[:, i + q - Q0]

where x_cols[p, m] = x[128*m + p] is the signal in "transposed" column
layout (prepared on host) and W_q[p, r] = h[128*(q-Q0) + p - r + c].
The contraction (partition) dim is the tap offset; the moving dim packs
412-512 consecutive output blocks, so the PE runs dense matmuls at full
rate in float32r.

pha bands: two passes (corr with h, then with flip(h)) — exactly the
reference's zero-phase scheme; the 'SAME' zero-pad crop of the
intermediate at the global sequence edges is enforced via per-core 0/1
masks on its halo columns (one SPMD program serves all cores).

amp bands: single fused pass with g = autocorr(h) (257 taps), which
equals the two-pass result everywhere except the first/last 64 samples
of the GLOBAL sequence; those get an exact rank-64 correction
(precomputed 64x64 matrices applied to x's global head/tail, fed as a
per-core input that is zero except on cores 0/7 — again SPMD-uniform).
"""
import numpy as np

import concourse.bass as bass
import concourse.tile as tile
from concourse import bacc, mybir
from concourse import bass_utils

# ---- problem geometry (hardcoded per contest rules) ----
T = 2097152
NCORES = 8
L = T // NCORES          # 262144 samples per core
LC = L // 128            # 2048 output columns per core
XH = 8                   # x halo columns each side
XC = LC + 2 * XH         # 2064 x columns
YH = 4                   # y1 halo columns each side (pha)
YC = LC + 2 * YH         # 2056 y1 columns
NB = 10                  # bands per filter group
KP, QP, Q0P = 769, 7, 3  # pha: taps, Toeplitz chunks, chunk offset
KA = 129                 # amp taps
QG, Q0G = 3, 1           # fused amp autocorr (257 taps): chunks, offset
CA = (KA - 1) // 2       # 64: amp edge-correction width
P1N = 412                # pass-1 matmul moving width (ceil(2056/5))
P2N = 512                # pass-2 / fused matmul moving width

F32 = mybir.dt.float32
F32R = mybir.dt.float32r


def _toeplitz_chunks(h, Q0, NQ):
    """W[q][p, r] = h[128*(q - Q0) + p - r + c], zero outside [0, K)."""
    h = np.asarray(h, np.float64)
    K = len(h)
    c = (K - 1) // 2
    W = np.zeros((NQ, 128, 128), np.float64)
    p = np.arange(128)[:, None]
    r = np.arange(128)[None, :]
    for q in range(NQ):
        k = 128 * (q - Q0) + p - r + c
        valid = (k >= 0) & (k < K)
        W[q][valid] = h[np.clip(k, 0, K - 1)][valid]
    return W.astype(np.float32)


def _amp_corr_lhsT(h):
    """Block-diag [MleftT, MrightT] edge-correction matrix for one amp band.

    ref[n] = fused[n] - Mleft[n,:] @ x[:64]        for n in [0, 64)
    ref[n] = fused[n] - Mright[n-T+64,:] @ x[T-64:] for n in [T-64, T)
    """
    h = np.asarray(h, np.float64)
    K = len(h)
    c = (K - 1) // 2

    def hpad(idx):
        v = np.zeros(idx.shape)
        ok = (idx >= 0) & (idx < K)
        v[ok] = h[np.clip(idx, 0, K - 1)][ok]
        return v

    n = np.arange(c)[:, None, None]
    i = np.arange(c)[None, :, None]
    m = np.arange(-c, 0)[None, None, :]
    Mleft = (hpad(n + c - m) * hpad(i + c - m)).sum(-1)
    mm = np.arange(1, c + 1)[None, None, :]
    Mright = (hpad(n - mm + 1) * hpad(i - mm + 1)).sum(-1)

    lhsT = np.zeros((128, 128), np.float64)
    lhsT[:c, :c] = Mleft.T          # lhsT[p=i, r=n]
    lhsT[c:2 * c, c:2 * c] = Mright.T
    return lhsT.astype(np.float32)


def _build_program():
    nc = bacc.Bacc("TRN2", target_bir_lowering=False, debug=False,
                   enable_asserts=True, num_devices=NCORES)

    x_ap = nc.dram_tensor("xT", [128, XC], F32R, kind="ExternalInput").ap()
    wp1_ap = nc.dram_tensor("wp1", [128, NB * QP * 128], F32R,
                            kind="ExternalInput").ap()
    wp2_ap = nc.dram_tensor("wp2", [128, NB * QP * 128], F32R,
                            kind="ExternalInput").ap()
    wg_ap = nc.dram_tensor("wg", [128, NB * QG * 128], F32R,
                           kind="ExternalInput").ap()
    wc_ap = nc.dram_tensor("wc", [128, NB * 128], F32,
                           kind="ExternalInput").ap()
    xe_ap = nc.dram_tensor("xe", [128, 1], F32, kind="ExternalInput").ap()
    ml_ap = nc.dram_tensor("mask_l", [128, YH], F32R, kind="ExternalInput").ap()
    mr_ap = nc.dram_tensor("mask_r", [128, YH], F32R, kind="ExternalInput").ap()
    out_ap = nc.dram_tensor("out", [2 * NB, 128, LC], F32,
                            kind="ExternalOutput").ap()

    with tile.TileContext(nc) as tc:
        with tc.tile_pool(name="const", bufs=1) as cpool, \
             tc.tile_pool(name="y1", bufs=2) as y1_pool, \
             tc.tile_pool(name="psum", bufs=6, space="PSUM") as psum_pool, \
             tc.tile_pool(name="psumc", bufs=2, space="PSUM") as psumc_pool, \
             tc.tile_pool(name="stage", bufs=4) as stage_pool:

            xt = cpool.tile([128, XC], F32R, name="xt", tag="x")
            wp1 = cpool.tile([128, NB * QP * 128], F32R, name="wp1t", tag="wp1")
            wp2 = cpool.tile([128, NB * QP * 128], F32R, name="wp2t", tag="wp2")
            wg = cpool.tile([128, NB * QG * 128], F32R, name="wgt", tag="wg")
            wc = cpool.tile([128, NB * 128], F32, name="wct", tag="wc")
            xe = cpool.tile([128, 1], F32, name="xet", tag="xe")
            ml = cpool.tile([128, YH], F32R, name="mlt", tag="mask_l")
            mr = cpool.tile([128, YH], F32R, name="mrt", tag="mask_r")

            # DMAs ordered by first use: x + band-0 weights + masks first,
            # so the PE never waits on the bulk of the weight traffic.
            # xT lands in 4 chunks so pass1(0) group 0 only waits for the
            # columns it reads.
            # xT on the ACT HWDGE ring, weights on the SP ring — the two
            # rings drain in parallel, halving time-to-first-matmul
            for s in range(0, XC, 516):
                e = min(XC, s + 516)
                nc.scalar.dma_start(xt[:, s:e], x_ap[:, s:e])

            def wslice(tile_, ap, b, nq):
                s = b * nq * 128
                e = (b + 1) * nq * 128
                nc.sync.dma_start(tile_[:, s:e], ap[:, s:e])

            wslice(wp1, wp1_ap, 0, QP)
            nc.sync.dma_start(ml[:], ml_ap[:])
            nc.sync.dma_start(mr[:], mr_ap[:])
            wslice(wp2, wp2_ap, 0, QP)
            nc.sync.dma_start(xe[:], xe_ap[:])
            wslice(wg, wg_ap, 0, QG)
            wslice(wc, wc_ap, 0, 1)
            for b in range(1, NB):
                wslice(wp1, wp1_ap, b, QP)
                wslice(wp2, wp2_ap, b, QP)
                wslice(wg, wg_ap, b, QG)
                wslice(wc, wc_ap, b, 1)

            y1_tiles = [None] * NB

            def pha_pass1(b):
                y1 = y1_pool.tile([128, YC], F32R, tag="y1")
                y1_tiles[b] = y1
                for g in range(5):
                    j0 = g * P1N
                    n = min(P1N, YC - j0)
                    ps = psum_pool.tile([128, P2N], F32, tag="ps")
                    for q in range(QP):
                        m0 = j0 + q + XH - YH - Q0P
                        nc.tensor.matmul(
                            ps[:, :n],
                            wp1[:, (b * QP + q) * 128:(b * QP + q + 1) * 128],
                            xt[:, m0:m0 + n],
                            start=(q == 0), stop=(q == QP - 1),
                        )
                    nc.vector.tensor_copy(y1[:, j0:j0 + n], ps[:, :n])
                # 'SAME' crop of y1 outside the global [0, T) range
                nc.vector.tensor_mul(y1[:, :YH], y1[:, :YH], ml[:])
                nc.vector.tensor_mul(y1[:, YC - YH:], y1[:, YC - YH:], mr[:])

            def pha_pass2(b):
                y1 = y1_tiles[b]
                for g in range(LC // P2N):
                    i0 = g * P2N
                    ps = psum_pool.tile([128, P2N], F32, tag="ps")
                    for q in range(QP):
                        j0 = i0 + q - Q0P + YH
                        nc.tensor.matmul(
                            ps[:],
                            wp2[:, (b * QP + q) * 128:(b * QP + q + 1) * 128],
                            y1[:, j0:j0 + P2N],
                            start=(q == 0), stop=(q == QP - 1),
                        )
                    st = stage_pool.tile([128, P2N], F32, tag="st")
                    if g % 2 == 0:
                        nc.vector.tensor_copy(st[:], ps[:])
                    else:
                        nc.scalar.copy(st[:], ps[:])
                    nc.sync.dma_start(out_ap[b, :, i0:i0 + P2N], st[:])

            def amp_band(b):
                # rank-64 global-edge correction (zero on inner cores)
                pc = psumc_pool.tile([128, 1], F32, tag="pc")
                nc.tensor.matmul(pc[:], wc[:, b * 128:(b + 1) * 128], xe[:],
                                 start=True, stop=True)
                for g in range(LC // P2N):
                    i0 = g * P2N
                    ps = psum_pool.tile([128, P2N], F32, tag="ps")
                    for q in range(QG):
                        m0 = i0 + q - Q0G + XH
                        nc.tensor.matmul(
                            ps[:],
                            wg[:, (b * QG + q) * 128:(b * QG + q + 1) * 128],
                            xt[:, m0:m0 + P2N],
                            start=(q == 0), stop=(q == QG - 1),
                        )
                    st = stage_pool.tile([128, P2N], F32, tag="st")
                    # alternate PSUM->SBUF drains across DVE and ACT so the
                    # copies keep up with the 3-matmul amp groups
                    if g % 2 == 0:
                        nc.vector.tensor_copy(st[:], ps[:])
                    else:
                        nc.scalar.copy(st[:], ps[:])
                    if g == 0:
                        nc.vector.tensor_sub(st[:CA, :1], st[:CA, :1],
                                             pc[:CA, :])
                    if g == LC // P2N - 1:
                        nc.vector.tensor_sub(st[CA:2 * CA, P2N - 1:],
                                             st[CA:2 * CA, P2N - 1:],
                                             pc[CA:2 * CA, :])
                    nc.sync.dma_start(out_ap[NB + b, :, i0:i0 + P2N], st[:])

            # pha bands software-pipelined so the PE never waits on the
            # pass1 -> pass2 handoff; amp bands interleaved between pha
            # bands so their drain-heavy PSUM copies land in regions
            # where DVE/ACT otherwise have slack.
            pha_pass1(0)
            for b in range(NB):
                if b + 1 < NB:
                    pha_pass1(b + 1)
                pha_pass2(b)
                amp_band(b)

    nc.compile()
    return nc


_CACHE = {}


def _get_program():
    if "nc" not in _CACHE:
        _CACHE["nc"] = _build_program()
    return _CACHE["nc"]


def _host_inputs(x, pha_filters, amp_filters):
    x = np.ascontiguousarray(np.asarray(x, np.float32).reshape(T))
    pha = np.asarray(pha_filters, np.float32)
    amp = np.asarray(amp_filters, np.float32)

    wp1 = np.stack([_toeplitz_chunks(h, Q0P, QP) for h in pha])
    wp2 = np.stack([_toeplitz_chunks(h[::-1], Q0P, QP) for h in pha])
    gs = [np.correlate(np.asarray(h, np.float64),
                       np.asarray(h, np.float64), "full") for h in amp]
    wg = np.stack([_toeplitz_chunks(g, Q0G, QG) for g in gs])
    wc = np.stack([_amp_corr_lhsT(h) for h in amp])  # (NB, 128, 128)

    def wlay(W):  # (NB, NQ, 128p, 128r) -> (128p, NB*NQ*128r)
        return np.ascontiguousarray(W.transpose(2, 0, 1, 3).reshape(128, -1))

    wp1, wp2, wg = wlay(wp1), wlay(wp2), wlay(wg)
    wc = np.ascontiguousarray(wc.transpose(1, 0, 2).reshape(128, -1))

    xp = np.zeros(T + 2 * XH * 128, np.float32)
    xp[XH * 128: XH * 128 + T] = x

    ones = np.ones((128, YH), np.float32)
    zeros = np.zeros((128, YH), np.float32)
    xe0 = np.zeros((128, 1), np.float32)
    xe_head = xe0.copy()
    xe_head[:CA, 0] = x[:CA]
    xe_tail = xe0.copy()
    xe_tail[CA:2 * CA, 0] = x[T - CA:]

    in_maps = []
    for c in range(NCORES):
        n0 = c * L
        xT = np.ascontiguousarray(
            xp[n0:n0 + L + 2 * XH * 128].reshape(XC, 128).T)
        in_maps.append({
            "xT": xT,
            "wp1": wp1, "wp2": wp2, "wg": wg, "wc": wc,
            "xe": xe_head if c == 0 else (xe_tail if c == NCORES - 1 else xe0),
            "mask_l": zeros if c == 0 else ones,
            "mask_r": zeros if c == NCORES - 1 else ones,
        })
    return in_maps


def _gather(results):
    out = np.empty((2 * NB, T), np.float32)
    for c in range(NCORES):
        oc = results[c]["out"]  # (20, 128, LC): [band, r, i] = y[128*i + r]
        out[:, c * L:(c + 1) * L] = oc.transpose(0, 2, 1).reshape(2 * NB, L)
    return out.reshape(1, 1, 2 * NB * T)


def run(x, pha_filters, amp_filters, trace=False):
    nc = _get_program()
    in_maps = _host_inputs(x, pha_filters, amp_filters)
    res = bass_utils.run_bass_kernel_spmd(
        nc, in_maps, core_ids=list(range(NCORES)), trace=trace)
    return _gather(res.results), res


def kernel(x, pha_filters, amp_filters):
    out, _ = run(x, pha_filters, amp_filters)
    return out
